# revision 53
# baseline (speedup 1.0000x reference)
"""GQA attention Trainium2 kernel (8 NeuronCores, SPMD, no collectives).

Sharding: 2-way data parallel (batch) x 4-way tensor parallel (heads).
Core c handles batch b=c//4 and head-group g=c%4 (8 q heads, 2 kv heads).
Each core produces a partial o_proj output (transposed, [HID, S] bf16);
the host sums the 4 partials per batch and transposes back.

V4: attention restructured for PE density + ACT relief:
 - causal trim: diagonal key-chunks only compute the valid query range
   (scores MM, exp, mask, PV all N-trimmed);
 - duo-pair processing: kv-head 0 (PE rows 0-63) and kv-head 1 (rows
   64-127) score-matmuls are emitted back-to-back so the PE row-tiling
   hardware runs them concurrently (tile_position auto-derived);
 - softmax exp fused over the kv-pair's two PSUM banks: one ACTIVATE
   per (chunk, q-head) on [128, 2*N'] instead of two;
 - masking only on the 128-wide diagonal stripe;
 - proj(sb+1) / oproj(sb-1) / gate(sb) work is split into units and
   interleaved into the attention chunk loop as PE filler so the
   in-order PE queue never starves while ACT runs the exps.
PSUM: mm(2) + sc(2x2) + av(2x1) = 8 banks.
"""

import os
import sys
import numpy as np

for _p in ("/opt/trn_rl_repo", "/root/.axon_site/_ro/trn_rl_repo"):
    if os.path.isdir(_p) and _p not in sys.path:
        sys.path.insert(0, _p)

import ml_dtypes

B, S, HID = 2, 2048, 2048
NH, NKV, HD = 32, 8, 64
ROPE = 32
EPS = 1e-6
SCALE = HD ** -0.5
NCORES = 8
QH = NH // 4      # 8 q heads per core
KVH = NKV // 4    # 2 kv heads per core
QD = QH * HD      # 512 per-core q dim
KD = KVH * HD     # 128 per-core kv dim
KC = HID // 128   # 16 contraction chunks
SB = S // 512     # 4 sequence blocks of 512
BF16 = ml_dtypes.bfloat16
MAGIC = 0x5F3759DF

# Schraudolph bf16 exp constants (policy-gated; unused when EXP_DVE empty)
EXP_A = 128.0 * 1.4426950408889634 * SCALE
EXP_B = 16256.0 - 7.41
# which chunk slots use DVE exp: set of (t % EXP_DVE_MOD) values
EXP_DVE = set(int(x) for x in os.environ.get("KERNEL_EXP_DVE", "").split(",")
              if x != "")

_CACHE = {}

SEL10 = np.zeros((10, 5, 128), np.float32)
for _c in range(5):
    SEL10[2 * _c, _c, 0:64] = 1
    SEL10[2 * _c + 1, _c, 64:128] = 1
SEL10 = SEL10.reshape(10, 640).astype(BF16)


def _build_bass(debug_dump=False):
    import concourse.bass as bass
    from concourse import bacc, mybir, tile
    from concourse.alu_op_type import AluOpType

    f32 = mybir.dt.float32
    bf16 = mybir.dt.bfloat16
    i32 = mybir.dt.int32
    i16 = mybir.dt.int16

    nc = bacc.Bacc("TRN2", target_bir_lowering=False, debug=False,
                   enable_asserts=False, num_devices=NCORES)

    hT = nc.dram_tensor("hT", [HID, S], bf16, kind="ExternalInput").ap()
    wqT = nc.dram_tensor("wqT", [HID, QD], bf16, kind="ExternalInput").ap()
    wkT = nc.dram_tensor("wkT", [HID, KD], bf16, kind="ExternalInput").ap()
    wvT = nc.dram_tensor("wvT", [HID, KD], bf16, kind="ExternalInput").ap()
    wgT = nc.dram_tensor("wgT", [HID, QD], bf16, kind="ExternalInput").ap()
    woT = nc.dram_tensor("woT", [QD, HID], bf16, kind="ExternalInput").ap()
    csAq = nc.dram_tensor("csAq", [128, S], bf16, kind="ExternalInput").ap()
    csBq = nc.dram_tensor("csBq", [128, S], bf16, kind="ExternalInput").ap()
    csAk = nc.dram_tensor("csAk", [128, S], bf16, kind="ExternalInput").ap()
    csBk = nc.dram_tensor("csBk", [128, S], bf16, kind="ExternalInput").ap()
    sel10d = nc.dram_tensor("sel10d", [10, 640], bf16,
                            kind="ExternalInput").ap()
    outT = nc.dram_tensor("outT", [HID, S], bf16, kind="ExternalOutput").ap()
    if debug_dump:
        dbg = {n: nc.dram_tensor(f"dbg_{n}", shp, mybir.dt.bfloat16,
                                 kind="ExternalOutput").ap()
               for n, shp in [("q", [128, 4, S]), ("k", [128, S]),
                              ("g", [128, 4, S]), ("v", [128, KC, KVH, HD + 1]),
                              ("og", [128, 4, S]), ("rstd", [16, 512]),
                              ("t64", [128, 8, 512]), ("rbv", [128, 8, 512]),
                              ("t2", [128, 8, 512]),
                              ("ogearly", [128, 4, 512])]}

    Exp = mybir.ActivationFunctionType.Exp
    Tanh = mybir.ActivationFunctionType.Tanh
    PSUM = bass.MemorySpace.PSUM
    # stream_shuffle mask: swap 16-row halves within each 32-row group
    ROT_MASK = list(range(16, 32)) + list(range(16))

    with tile.TileContext(nc) as tc:
        with tc.tile_pool(name="persist", bufs=1) as pp, \
             tc.tile_pool(name="hblk", bufs=3) as hp, \
             tc.tile_pool(name="rope", bufs=2) as rp, \
             tc.tile_pool(name="qa", bufs=5) as qap, \
             tc.tile_pool(name="sqp", bufs=1) as sqp, \
             tc.tile_pool(name="probs", bufs=6) as prp, \
             tc.tile_pool(name="attsm", bufs=2) as asm, \
             tc.tile_pool(name="attsm2", bufs=2) as asm2, \
             tc.tile_pool(name="ostg", bufs=2) as ostg, \
             tc.tile_pool(name="mm", bufs=2, space=PSUM) as mmp, \
             tc.tile_pool(name="sc", bufs=2, space=PSUM) as scp, \
             tc.tile_pool(name="av", bufs=2, space=PSUM) as avp:

            # ---------------- persistent state ----------------
            qT_sb = pp.tile([128, 4, S], bf16)       # roped+normed q
            kT_sb = pp.tile([128, S], bf16)          # roped+normed k
            g_sb = pp.tile([128, 4, S], bf16)        # sigmoid(gate)
            v_sb = pp.tile([128, KC, KVH, HD + 1], bf16)  # natural V + ones
            og_sb = pp.tile([128, 4, S], bf16)       # gated attn out
            wo_sb = pp.tile([128, 4, KC, 128], bf16)
            wq_sb = pp.tile([128, KC, QD], bf16)
            wk_sb = pp.tile([128, KC, KD], bf16)
            wv_sb = pp.tile([128, KC, KD], bf16)
            wg_sb = pp.tile([128, KC, QD], bf16)
            csA_q = pp.tile([128, S], bf16)
            csB_q = pp.tile([128, S], bf16)
            csA_k = pp.tile([128, S], bf16)
            csB_k = pp.tile([128, S], bf16)

            # first block's activations + q-path weights first so the PE
            # can start ASAP; h blocks ride the scalar queue so they don't
            # serialize behind the weight DMAs on sync.
            ha0 = hp.tile([128, 8, 512], bf16, tag="hblk")
            hb0 = hp.tile([128, 8, 512], bf16, tag="hblk")
            wq_view = wqT.rearrange("(c p) m -> p c m", p=128)
            nc.sync.dma_start(out=wq_sb[:, :, 0:128],
                              in_=wq_view[:, :, 0:128])
            for ho in range(2):
                nc.scalar.dma_start(
                    out=ha0[:, 4 * ho:4 * ho + 4, :],
                    in_=hT[512 * ho:512 * ho + 512, 0:512].rearrange(
                        "(c p) s -> p c s", p=128))
            for ho in range(2):
                nc.scalar.dma_start(
                    out=hb0[:, 4 * ho:4 * ho + 4, :],
                    in_=hT[1024 + 512 * ho:1536 + 512 * ho, 0:512].rearrange(
                        "(c p) s -> p c s", p=128))
            for m in range(1, 4):
                nc.sync.dma_start(out=wq_sb[:, :, m * 128:(m + 1) * 128],
                                  in_=wq_view[:, :, m * 128:(m + 1) * 128])
            nc.sync.dma_start(out=wk_sb,
                              in_=wkT.rearrange("(c p) m -> p c m", p=128))
            nc.sync.dma_start(out=csA_q, in_=csAq)
            nc.sync.dma_start(out=csB_q, in_=csBq)
            nc.sync.dma_start(out=csA_k, in_=csAk)
            nc.sync.dma_start(out=csB_k, in_=csBk)
            nc.sync.dma_start(out=wv_sb,
                              in_=wvT.rearrange("(c p) m -> p c m", p=128))
            nc.sync.dma_start(out=wg_sb,
                              in_=wgT.rearrange("(c p) m -> p c m", p=128))
            nc.sync.dma_start(out=wo_sb,
                              in_=woT.rearrange("(c p) (mb mm) -> p c mb mm",
                                                p=128, mm=128))

            ident = pp.tile([128, 128], bf16)
            from concourse.masks import make_identity
            make_identity(nc, ident)
            # ones10[:, c, :]: gather selector -- sumsq of block c's two
            # 64-row halves lands on psum partitions 2c / 2c+1 directly.
            ones10 = pp.tile([128, 5, 10], bf16)
            nc.vector.memset(ones10, 0.0)
            for _c in range(5):
                nc.vector.memset(ones10[0:64, _c, 2 * _c:2 * _c + 1], 1.0)
                nc.vector.memset(ones10[64:128, _c, 2 * _c + 1:2 * _c + 2],
                                 1.0)
            # sel10[:, c, :]: broadcast selector -- rb_ps rows 0:64 get
            # rstd_bf[2c], rows 64:128 get rstd_bf[2c+1] (K=10 matmul).
            # Loaded from DRAM: engine writes can't start at odd partitions.
            sel10 = pp.tile([10, 5, 128], bf16)
            nc.sync.dma_start(out=sel10,
                              in_=sel10d.rearrange("p (c m) -> p c m", m=128))
            nc.vector.memset(v_sb[:, :, :, HD:HD + 1], 1.0)
            if debug_dump:
                nc.vector.memset(og_sb, 777.0)

            # per-sb rstd state (persistent; reused each sb serially)
            sqg = pp.tile([16, 512], f32)            # partition-major sumsq
            rstd_bf = pp.tile([16, 512], bf16)       # final 8/sqrt(ms)

            def proj_chunk(ha, hb, w_sb, msl):
                ps = mmp.tile([128, 512], f32, tag="mm")
                for kc in range(KC):
                    h = ha if kc < 8 else hb
                    nc.tensor.matmul(ps, w_sb[:, kc, msl],
                                     h[:, kc % 8, :],
                                     start=(kc == 0), stop=(kc == KC - 1))
                return ps

            Square = mybir.ActivationFunctionType.Square
            dgather = []   # deferred sumsq-gather closures (see rope_block)

            def flush_dg():
                while dgather:
                    dgather.pop(0)()

            def rope_block(ps, csA, csB, c):
                """bf16 rope on sbuf; defers the sumsq gather one proj
                chain so the PE never waits on the ACT square.  The gather
                matmul reuses rows 0:2 of the spent ps bank (data already
                consumed by raw/sq), keeping the mm-pool rotation intact."""
                raw = rp.tile([128, 512], bf16, tag="raw")
                nc.scalar.copy(out=raw, in_=ps)
                sq = rp.tile([128, 512], bf16, tag="sq", bufs=1)
                nc.scalar.activation(out=sq, in_=ps, func=Square)

                def gather():
                    # selector matmul: block c's sumsq on psum rows 2c:2c+2,
                    # zeros elsewhere; accumulate into sqg rows 0:10 (c=0
                    # initializes and adds the EPS bias for every row).
                    nc.tensor.matmul(ps[0:10, :], ones10[:, c, :], sq,
                                     start=True, stop=True)
                    if c == 0:
                        nc.vector.tensor_scalar(out=sqg[0:10, :],
                                                in0=ps[0:10, :],
                                                scalar1=float(HD) * EPS,
                                                scalar2=None,
                                                op0=AluOpType.add)
                    else:
                        nc.vector.tensor_tensor(out=sqg[0:10, :],
                                                in0=sqg[0:10, :],
                                                in1=ps[0:10, :],
                                                op=AluOpType.add)
                dgather.append(gather)
                rot = rp.tile([128, 512], bf16, tag="rot", bufs=1)
                nc.vector.stream_shuffle(rot, raw, ROT_MASK)
                t1 = rp.tile([128, 512], bf16, tag="t1")
                nc.vector.tensor_tensor(out=t1, in0=raw, in1=csA,
                                        op=AluOpType.mult)
                t2 = rp.tile([128, 512], bf16, tag="t2")
                nc.vector.tensor_tensor(out=t2, in0=rot, in1=csB,
                                        op=AluOpType.mult)
                qa2 = qap.tile([128, 512], bf16, tag="qa2")
                nc.vector.tensor_add(qa2, t1, t2)
                return qa2

            def rstd_chain():
                """sqg[0:10] -> rstd_bf[0:10] (=8/sqrt(ms), bf16)"""
                ms = sqg[0:10, :]
                ms_i = ms.bitcast(i32)
                sh = sqp.tile([16, 512], i32, tag="sh")
                nc.vector.tensor_scalar(out=sh[0:10], in0=ms_i, scalar1=1,
                                        scalar2=None,
                                        op0=AluOpType.logical_shift_right)
                y = sqp.tile([16, 512], f32, tag="y")
                y_i = y.bitcast(i32)
                # y0 bits = MAGIC - (i >> 1)
                nc.vector.tensor_scalar(out=y_i[0:10], in0=sh[0:10],
                                        scalar1=-1, scalar2=MAGIC,
                                        op0=AluOpType.mult,
                                        op1=AluOpType.add)
                for _ in range(2):   # Newton iterations for rsqrt
                    a = sqp.tile([16, 512], f32, tag="nra")
                    nc.vector.tensor_tensor(out=a[0:10], in0=y[0:10],
                                            in1=y[0:10], op=AluOpType.mult)
                    nc.vector.tensor_tensor(out=a[0:10], in0=a[0:10],
                                            in1=ms, op=AluOpType.mult)
                    nc.vector.tensor_scalar(out=a[0:10], in0=a[0:10],
                                            scalar1=-0.5, scalar2=1.5,
                                            op0=AluOpType.mult,
                                            op1=AluOpType.add)
                    nc.vector.tensor_tensor(out=y[0:10], in0=y[0:10],
                                            in1=a[0:10], op=AluOpType.mult)
                nc.vector.tensor_scalar(out=rstd_bf[0:10], in0=y[0:10],
                                        scalar1=float(HD) ** 0.5, scalar2=None,
                                        op0=AluOpType.mult)

            # -------- proj work units (emitted lazily as PE filler) --------
            def proj_units(sb, ha, hb):
                """Full qkv/gate proj + rstd for block sb, as closures.
                Gate chains sit between rstd_chain and the applies so the
                serial rstd DVE chain hides behind ~11us of PE work."""
                s0 = sb * 512
                ssl = slice(s0, s0 + 512)
                qa_list = [None] * 4
                ka2_box = [None]

                def uq(m):
                    def f():
                        ps = proj_chunk(ha, hb, wq_sb,
                                        slice(m * 128, (m + 1) * 128))
                        flush_dg()
                        qa_list[m] = rope_block(ps, csA_q[:, ssl],
                                                csB_q[:, ssl], m)
                    return f

                def uk():
                    psk = proj_chunk(ha, hb, wk_sb, slice(0, 128))
                    flush_dg()
                    ka2_box[0] = rope_block(psk, csA_k[:, ssl],
                                            csB_k[:, ssl], 4)

                def uv():
                    psv = proj_chunk(ha, hb, wv_sb, slice(0, 128))
                    flush_dg()
                    vt = rp.tile([128, 512], bf16, tag="t2")
                    nc.vector.tensor_copy(out=vt, in_=psv)
                    for ss in range(4):
                        tp = mmp.tile([128, 128], bf16, tag="mm")
                        nc.tensor.transpose(tp, vt[:, ss * 128:(ss + 1) * 128],
                                            ident)
                        chunk = sb * 4 + ss
                        nc.vector.tensor_copy(out=v_sb[:, chunk, 0, 0:HD],
                                              in_=tp[:, 0:64])
                        nc.vector.tensor_copy(out=v_sb[:, chunk, 1, 0:HD],
                                              in_=tp[:, 64:128])

                def ug(m):
                    def f():
                        ps = proj_chunk(ha, hb, wg_sb,
                                        slice(m * 128, (m + 1) * 128))
                        th = rp.tile([128, 512], bf16, tag="t1")
                        nc.scalar.activation(out=th, in_=ps, func=Tanh,
                                             scale=0.5)
                        nc.vector.tensor_scalar(out=g_sb[:, m, ssl], in0=th,
                                                scalar1=0.5, scalar2=0.5,
                                                op0=AluOpType.mult,
                                                op1=AluOpType.add)
                    return f

                def uapply(c):
                    def f():
                        rb_ps = mmp.tile([128, 512], f32, tag="mm")
                        nc.tensor.matmul(rb_ps, sel10[:, c, :],
                                         rstd_bf[0:10, :],
                                         start=True, stop=True)
                        if c < 4:
                            r = (c // 2) * 64
                            cb = 2 * (c % 2)
                            nc.vector.tensor_tensor(
                                out=qT_sb[r:r + 64, cb, ssl],
                                in0=qa_list[c][0:64, :], in1=rb_ps[0:64, :],
                                op=AluOpType.mult)
                            nc.vector.tensor_tensor(
                                out=qT_sb[r:r + 64, cb + 1, ssl],
                                in0=qa_list[c][64:128, :],
                                in1=rb_ps[64:128, :],
                                op=AluOpType.mult)
                        else:
                            nc.vector.tensor_tensor(out=kT_sb[:, ssl],
                                                    in0=ka2_box[0],
                                                    in1=rb_ps,
                                                    op=AluOpType.mult)
                    return f

                # pre: everything attn(sb) slot 0 depends on (kT, qT, v);
                # uv hosts the k-block's deferred gather; g0/g2/g1 hide the
                # serial rstd chain before the applies.  late: g3 (needed
                # only by the p=1 drains) gives the attention tail PE work.
                pre = [uq(0), uq(1), uq(2), uq(3), uk, uv, rstd_chain,
                       ug(0), ug(2), ug(1), uapply(4)]
                pre += [uapply(c) for c in range(4)]
                late = [ug(3)]
                return pre, late

            def oproj_units(sb):
                ssl = slice(sb * 512, (sb + 1) * 512)

                def uo(m):
                    def f():
                        po = mmp.tile([128, 512], f32, tag="mm")
                        for oc in range(4):
                            nc.tensor.matmul(po, wo_sb[:, oc, m, :],
                                             og_sb[:, oc, ssl],
                                             start=(oc == 0), stop=(oc == 3))
                        stg = ostg.tile([128, 512], bf16, tag="stg")
                        if m % 2 == 0:
                            nc.scalar.copy(out=stg, in_=po)
                        else:
                            nc.vector.tensor_copy(out=stg, in_=po)
                        nc.sync.dma_start(
                            out=outT[m * 128:(m + 1) * 128, ssl],
                            in_=stg)
                    return f
                return [uo(m) for m in range(KC)]

            # ---------------- attention (per seq block) ----------------
            def attn_sb(sb, early, late):
                """Flat slot stream over 4 (duo-pair, q-head) passes with
                causal trim + fused exp.  AV lags scores by LAG slots and
                carries across pass boundaries; softmax drains are deferred
                into the next pass so the PE pipeline never collapses at a
                pass edge.  early: fillers whose results the next attn
                block needs immediately -- paced to finish a few slots
                before the end; late: dependency-free fillers (oproj) paced
                through the flush/drain tail."""
                s0 = sb * 512
                nkc = 4 * (sb + 1)
                LAG = 2
                passes = [(p, hh) for p in (0, 1) for hh in (0, 1)]
                total = 4 * nkc
                flush = 4
                ne, nl = len(early), len(late)
                eden = max(1, total - 4)
                state = {"slot": 0, "efill": 0, "lfill": 0}
                pstate = {}
                pend_av = []      # (due_slot, pass_idx, t) FIFO
                pend_drain = []   # (due_slot, pass_idx, kvi, phase) FIFO

                def pace():
                    state["slot"] += 1
                    # ceil pacing: a short early list must still empty well
                    # before its in-loop consumers (g3 feeds the p=1 drains)
                    want = min(ne, -(-ne * state["slot"] // eden))
                    while state["efill"] < want:
                        early[state["efill"]]()
                        state["efill"] += 1
                    # late list: start immediately (covers the previous
                    # block's drain tail) but hold a couple of units back
                    # for this block's own drain tail.
                    want = min(nl, -(-nl * state["slot"] // (total + flush + 8)))
                    while state["lfill"] < want:
                        late[state["lfill"]]()
                        state["lfill"] += 1

                def emit_av(pi, t):
                    st = pstate[pi]
                    tl = t - 4 * sb
                    qoff = 128 * tl if tl > 0 else 0
                    qsl = slice(qoff, 512)
                    pr = st["probs"].pop(t)
                    for kvi in range(2):
                        nc.tensor.matmul(
                            st["av"][kvi][:, qsl],
                            v_sb[:, t, kvi, :],
                            pr[:, kvi, qsl],
                            start=(t == 0), stop=(t == nkc - 1))

                def emit_drain1(pi, kvi):
                    """phase 1: the two av reads (cast-copy + psum-direct
                    recip) -- after these the av banks are reusable."""
                    st = pstate[pi]
                    av_t = st["av"][kvi]
                    p, hh = passes[pi]
                    rsl = slice(hh * 64, hh * 64 + 64)
                    t64 = asm2.tile([128, 512], bf16, tag="t")
                    nc.vector.tensor_copy(out=t64[rsl, :], in_=av_t[0:64, :])
                    dn = asm.tile([1, 512], f32, tag="dn", bufs=1)
                    nc.vector.tensor_copy(out=dn, in_=av_t[64:65, :])
                    recip = asm.tile([1, 512], f32, tag="recip")
                    nc.vector.reciprocal_approx_fast(out=recip, in_=dn)
                    st[("d1", kvi)] = (t64, recip)

                def emit_drain2(pi, kvi):
                    """phase 2: og = (av/denom)*gate off the critical path."""
                    p, hh = passes[pi]
                    st = pstate[pi]
                    t64, recip = st.pop(("d1", kvi))
                    duo = p + 2 * kvi
                    rsl = slice(hh * 64, hh * 64 + 64)
                    rb16 = asm.tile([1, 512], bf16, tag="rb16")
                    nc.vector.tensor_copy(out=rb16, in_=recip)
                    rbv = asm.tile([128, 512], bf16, tag="rbv")
                    nc.gpsimd.partition_broadcast(rbv, rb16)
                    t2 = asm2.tile([128, 512], bf16, tag="t2")
                    nc.vector.tensor_tensor(out=t2[rsl, :], in0=t64[rsl, :],
                                            in1=rbv[rsl, :],
                                            op=AluOpType.mult)
                    nc.vector.tensor_tensor(
                        out=og_sb[rsl, duo, s0:s0 + 512],
                        in0=t2[rsl, :],
                        in1=g_sb[rsl, duo, s0:s0 + 512],
                        op=AluOpType.mult)
                    if debug_dump and sb == SB - 1:
                        di = pi * 2 + kvi
                        nc.sync.dma_start(out=dbg["t64"][:, di, :], in_=t64)
                        nc.sync.dma_start(out=dbg["rbv"][:, di, :], in_=rbv)
                        nc.sync.dma_start(out=dbg["t2"][:, di, :], in_=t2)
                        nc.sync.dma_start(
                            out=dbg["ogearly"][rsl, duo, :],
                            in_=og_sb[rsl, duo, s0:s0 + 512])

                for s in range(total + flush):
                    if s < total:
                        pi, t = divmod(s, nkc)
                        if t == 0:
                            p, hh = passes[pi]
                            pstate[pi] = {
                                "qc": 2 * p + hh,
                                "probs": {},
                                "av": [avp.tile([65, 512], f32, tag="av",
                                                name=f"av_{pi}_{i}")
                                       for i in range(2)],
                            }
                        st = pstate[pi]
                        tl = t - 4 * sb
                        qoff = 128 * tl if tl > 0 else 0
                        qsl = slice(qoff, 512)
                        sc2 = scp.tile([128, 2, 512], f32, tag="sc")
                        for kvi in range(2):
                            r0 = 64 * kvi
                            nc.tensor.matmul(
                                sc2[:, kvi, qsl],
                                kT_sb[r0:r0 + 64, t * 128:(t + 1) * 128],
                                qT_sb[r0:r0 + 64, st["qc"],
                                      s0 + qoff:s0 + 512],
                                start=True, stop=True)
                        pr = prp.tile([128, 2, 512], bf16, tag="probs")
                        if (s % 4) in EXP_DVE:
                            nc.vector.tensor_scalar(
                                out=pr[:, :, qsl].bitcast(i16),
                                in0=sc2[:, :, qsl],
                                scalar1=EXP_A, scalar2=EXP_B,
                                op0=AluOpType.mult, op1=AluOpType.add)
                        else:
                            nc.scalar.activation(out=pr[:, :, qsl],
                                                 in_=sc2[:, :, qsl],
                                                 func=Exp, scale=SCALE)
                        if tl >= 0:
                            for kvi in range(2):
                                nc.gpsimd.affine_select(
                                    out=pr[:, kvi, qoff:qoff + 128],
                                    in_=pr[:, kvi, qoff:qoff + 128],
                                    compare_op=mybir.AluOpType.is_ge,
                                    fill=0.0, base=0, channel_multiplier=-1,
                                    pattern=[[1, 128]])
                        st["probs"][t] = pr
                        # first AVs of a pass wait two extra slots so the
                        # previous pass's drain reads land well before the
                        # av-bank start=True reuse (PE-W vs DVE-R hazard).
                        pend_av.append((s + LAG + (2 if t <= 1 else 0),
                                        pi, t))
                        if t == nkc - 1:
                            # drain phase 1 right after this pass's last AV
                            pend_drain.append((s + LAG + 1, pi, 0, 1))
                            pend_drain.append((s + LAG + 1, pi, 1, 1))
                            pend_drain.append((s + LAG + 2, pi, 0, 2))
                            pend_drain.append((s + LAG + 2, pi, 1, 2))
                    # deferred drains first (they free av banks), then AVs
                    while pend_drain and pend_drain[0][0] <= s:
                        _, pi2, kvi, ph = pend_drain.pop(0)
                        (emit_drain1 if ph == 1 else emit_drain2)(pi2, kvi)
                    while pend_av and pend_av[0][0] <= s:
                        _, pi2, t2 = pend_av.pop(0)
                        emit_av(pi2, t2)
                    pace()
                while pend_drain:
                    _, pi2, kvi, ph = pend_drain.pop(0)
                    (emit_drain1 if ph == 1 else emit_drain2)(pi2, kvi)
                while pend_av:
                    _, pi2, t2 = pend_av.pop(0)
                    emit_av(pi2, t2)
                while state["efill"] < ne:
                    early[state["efill"]]()
                    state["efill"] += 1
                while state["lfill"] < nl:
                    late[state["lfill"]]()
                    state["lfill"] += 1

            # ================= fused pipeline =================
            h_tiles = {0: (ha0, hb0)}
            pre0, late0 = proj_units(0, ha0, hb0)
            for u in pre0:
                u()
            late_units = {0: late0}
            for sb in range(SB):
                if sb < SB - 1:
                    s1 = (sb + 1) * 512
                    ha = hp.tile([128, 8, 512], bf16, tag="hblk")
                    hb = hp.tile([128, 8, 512], bf16, tag="hblk")
                    nc.scalar.dma_start(
                        out=ha,
                        in_=hT[0:1024, s1:s1 + 512].rearrange(
                            "(c p) s -> p c s", p=128))
                    nc.scalar.dma_start(
                        out=hb,
                        in_=hT[1024:2048, s1:s1 + 512].rearrange(
                            "(c p) s -> p c s", p=128))
                    h_tiles[sb + 1] = (ha, hb)
                early = list(late_units[sb])
                late = []
                if sb > 0:
                    late = oproj_units(sb - 1)
                if sb < SB - 1:
                    pre_n, late_n = proj_units(sb + 1, *h_tiles[sb + 1])
                    early = early + pre_n
                    late_units[sb + 1] = late_n
                attn_sb(sb, early, late)
            for u in oproj_units(SB - 1):
                u()

            if debug_dump:
                nc.sync.dma_start(out=dbg["q"], in_=qT_sb)
                nc.sync.dma_start(out=dbg["k"], in_=kT_sb)
                nc.sync.dma_start(out=dbg["g"], in_=g_sb)
                nc.sync.dma_start(out=dbg["v"], in_=v_sb)
                nc.sync.dma_start(out=dbg["og"], in_=og_sb)
                nc.sync.dma_start(out=dbg["rstd"], in_=rstd_bf)

    nc.compile()
    return nc


def _host_prep(hidden_states, cos, sin, Wq, Wk, Wv, Wg, Wo, q_norm_w, k_norm_w):
    """Build per-core input maps."""

    def cs_tables(cos_b, sin_b, w):
        # csA/csB [128, S]: row p -> head-local dim d = p % 64
        A = np.empty((128, S), np.float32)
        Bt = np.empty((128, S), np.float32)
        cosT = cos_b.T  # [32, S]
        sinT = sin_b.T
        for blk in (0, 64):
            A[blk + 0:blk + 32] = cosT * w[0:32, None]
            A[blk + 32:blk + 64] = w[32:64, None]
            Bt[blk + 0:blk + 16] = -sinT[0:16] * w[16:32, None]
            Bt[blk + 16:blk + 32] = sinT[16:32] * w[0:16, None]
            Bt[blk + 32:blk + 64] = 0.0
        return A.astype(BF16), Bt.astype(BF16)

    in_maps = []
    for c in range(NCORES):
        b, g = c // 4, c % 4
        qs = slice(g * QD, (g + 1) * QD)
        ks = slice(g * KD, (g + 1) * KD)
        csA_q, csB_q = cs_tables(cos[b], sin[b], np.asarray(q_norm_w))
        csA_k, csB_k = cs_tables(cos[b], sin[b], np.asarray(k_norm_w))
        in_maps.append({
            "hT": np.ascontiguousarray(hidden_states[b].T).astype(BF16),
            "wqT": np.ascontiguousarray(Wq[qs].T).astype(BF16),
            "wkT": np.ascontiguousarray(Wk[ks].T).astype(BF16),
            "wvT": np.ascontiguousarray(Wv[ks].T).astype(BF16),
            "wgT": np.ascontiguousarray(Wg[qs].T).astype(BF16),
            "woT": np.ascontiguousarray(Wo[:, qs].T).astype(BF16),
            "csAq": csA_q, "csBq": csB_q, "csAk": csA_k, "csBk": csB_k,
            "sel10d": SEL10,
        })
    return in_maps


def kernel(hidden_states, cos, sin, Wq, Wk, Wv, Wg, Wo, q_norm_w, k_norm_w):
    from concourse import bass_utils

    if "nc" not in _CACHE:
        _CACHE["nc"] = _build_bass()
    nc = _CACHE["nc"]

    in_maps = _host_prep(hidden_states, cos, sin, Wq, Wk, Wv, Wg, Wo,
                         q_norm_w, k_norm_w)

    trace = bool(int(os.environ.get("KERNEL_TRACE", "0")))
    kwargs = {}
    if trace:
        # the agent image's antenv lacks axon_hooks; recreate it from the
        # boot helper so run_bass_kernel_spmd(trace=True) can NTFF-profile
        try:
            import antenv.axon_hooks  # noqa: F401
        except ImportError:
            import types
            sys.path.insert(0, "/root/.axon_site")
            from trn_agent_boot.trn_boot import _ntff_profile_via_ctypes
            hook = _ntff_profile_via_ctypes("/opt/axon/libaxon_pjrt.so")
            mod = types.ModuleType("antenv.axon_hooks")
            mod.get_axon_ntff_profile_hook = lambda: hook
            sys.modules["antenv.axon_hooks"] = mod
        tmpdir = os.environ.get("KERNEL_TRACE_DIR") or None
        kwargs = dict(trace=True, tmpdir=tmpdir)
    res = bass_utils.run_bass_kernel_spmd(nc, in_maps,
                                          core_ids=list(range(NCORES)),
                                          **kwargs)
    if trace and res.exec_time_ns is not None:
        print(f"HW exec time: {res.exec_time_ns} ns")
        _CACHE["exec_time_ns"] = res.exec_time_ns

    out = np.zeros((B, S, HID), np.float32)
    for c in range(NCORES):
        b = c // 4
        out[b] += res.results[c]["outT"].astype(np.float32).T
    return out


if __name__ == "__main__":
    rng = np.random.default_rng(0)
    hs = rng.standard_normal((B, S, HID), dtype=np.float32)
    cos = rng.random((B, S, ROPE), dtype=np.float32)
    sin = rng.random((B, S, ROPE), dtype=np.float32)
    out = kernel(hidden_states=hs, cos=cos, sin=sin,
                 Wq=rng.standard_normal((NH * HD, HID), dtype=np.float32) * 0.02,
                 Wk=rng.standard_normal((NKV * HD, HID), dtype=np.float32) * 0.02,
                 Wv=rng.standard_normal((NKV * HD, HID), dtype=np.float32) * 0.02,
                 Wg=rng.standard_normal((NH * HD, HID), dtype=np.float32) * 0.02,
                 Wo=rng.standard_normal((HID, NH * HD), dtype=np.float32) * 0.02,
                 q_norm_w=np.ones(HD, np.float32),
                 k_norm_w=np.ones(HD, np.float32))
    print(out.shape, out.dtype)


# revision 54
# speedup vs baseline: 1.0183x; 1.0183x over previous
"""GQA attention Trainium2 kernel (8 NeuronCores, SPMD, no collectives).

Sharding: 2-way data parallel (batch) x 4-way tensor parallel (heads).
Core c handles batch b=c//4 and head-group g=c%4 (8 q heads, 2 kv heads).
Each core produces a partial o_proj output (transposed, [HID, S] bf16);
the host sums the 4 partials per batch and transposes back.

V4: attention restructured for PE density + ACT relief:
 - causal trim: diagonal key-chunks only compute the valid query range
   (scores MM, exp, mask, PV all N-trimmed);
 - duo-pair processing: kv-head 0 (PE rows 0-63) and kv-head 1 (rows
   64-127) score-matmuls are emitted back-to-back so the PE row-tiling
   hardware runs them concurrently (tile_position auto-derived);
 - softmax exp fused over the kv-pair's two PSUM banks: one ACTIVATE
   per (chunk, q-head) on [128, 2*N'] instead of two;
 - masking only on the 128-wide diagonal stripe;
 - proj(sb+1) / oproj(sb-1) / gate(sb) work is split into units and
   interleaved into the attention chunk loop as PE filler so the
   in-order PE queue never starves while ACT runs the exps.
PSUM: mm(2) + sc(2x2) + av(2x1) = 8 banks.
"""

import os
import sys
import numpy as np

for _p in ("/opt/trn_rl_repo", "/root/.axon_site/_ro/trn_rl_repo"):
    if os.path.isdir(_p) and _p not in sys.path:
        sys.path.insert(0, _p)

import ml_dtypes

B, S, HID = 2, 2048, 2048
NH, NKV, HD = 32, 8, 64
ROPE = 32
EPS = 1e-6
SCALE = HD ** -0.5
NCORES = 8
QH = NH // 4      # 8 q heads per core
KVH = NKV // 4    # 2 kv heads per core
QD = QH * HD      # 512 per-core q dim
KD = KVH * HD     # 128 per-core kv dim
KC = HID // 128   # 16 contraction chunks
SB = S // 512     # 4 sequence blocks of 512
BF16 = ml_dtypes.bfloat16
MAGIC = 0x5F3759DF

# Schraudolph bf16 exp constants (policy-gated; unused when EXP_DVE empty)
EXP_A = 128.0 * 1.4426950408889634 * SCALE
EXP_B = 16256.0 - 7.41
# which chunk slots use DVE exp: set of (t % EXP_DVE_MOD) values
EXP_DVE = set(int(x) for x in os.environ.get("KERNEL_EXP_DVE", "").split(",")
              if x != "")

_CACHE = {}

SEL10 = np.zeros((10, 5, 128), np.float32)
for _c in range(5):
    SEL10[2 * _c, _c, 0:64] = 1
    SEL10[2 * _c + 1, _c, 64:128] = 1
SEL10 = SEL10.reshape(10, 640).astype(BF16)


def _build_bass(debug_dump=False):
    import concourse.bass as bass
    from concourse import bacc, mybir, tile
    from concourse.alu_op_type import AluOpType

    f32 = mybir.dt.float32
    bf16 = mybir.dt.bfloat16
    i32 = mybir.dt.int32
    i16 = mybir.dt.int16

    nc = bacc.Bacc("TRN2", target_bir_lowering=False, debug=False,
                   enable_asserts=False, num_devices=NCORES)

    hT = nc.dram_tensor("hT", [HID, S], bf16, kind="ExternalInput").ap()
    wqT = nc.dram_tensor("wqT", [HID, QD], bf16, kind="ExternalInput").ap()
    wkT = nc.dram_tensor("wkT", [HID, KD], bf16, kind="ExternalInput").ap()
    wvT = nc.dram_tensor("wvT", [HID, KD], bf16, kind="ExternalInput").ap()
    wgT = nc.dram_tensor("wgT", [HID, QD], bf16, kind="ExternalInput").ap()
    woT = nc.dram_tensor("woT", [QD, HID], bf16, kind="ExternalInput").ap()
    csAq = nc.dram_tensor("csAq", [128, S], bf16, kind="ExternalInput").ap()
    csBq = nc.dram_tensor("csBq", [128, S], bf16, kind="ExternalInput").ap()
    csAk = nc.dram_tensor("csAk", [128, S], bf16, kind="ExternalInput").ap()
    csBk = nc.dram_tensor("csBk", [128, S], bf16, kind="ExternalInput").ap()
    sel10d = nc.dram_tensor("sel10d", [10, 640], bf16,
                            kind="ExternalInput").ap()
    outT = nc.dram_tensor("outT", [HID, S], bf16, kind="ExternalOutput").ap()
    if debug_dump:
        dbg = {n: nc.dram_tensor(f"dbg_{n}", shp, mybir.dt.bfloat16,
                                 kind="ExternalOutput").ap()
               for n, shp in [("q", [128, 4, S]), ("k", [128, S]),
                              ("g", [128, 4, S]), ("v", [128, KC, KVH, HD + 1]),
                              ("og", [128, 4, S]), ("rstd", [16, 512]),
                              ("t64", [128, 8, 512]), ("rbv", [128, 8, 512]),
                              ("t2", [128, 8, 512]),
                              ("ogearly", [128, 4, 512])]}

    Exp = mybir.ActivationFunctionType.Exp
    Tanh = mybir.ActivationFunctionType.Tanh
    PSUM = bass.MemorySpace.PSUM
    # stream_shuffle mask: swap 16-row halves within each 32-row group
    ROT_MASK = list(range(16, 32)) + list(range(16))

    with tile.TileContext(nc) as tc:
        with tc.tile_pool(name="persist", bufs=1) as pp, \
             tc.tile_pool(name="hblk", bufs=3) as hp, \
             tc.tile_pool(name="rope", bufs=2) as rp, \
             tc.tile_pool(name="qa", bufs=5) as qap, \
             tc.tile_pool(name="sqp", bufs=1) as sqp, \
             tc.tile_pool(name="probs", bufs=6) as prp, \
             tc.tile_pool(name="attsm", bufs=2) as asm, \
             tc.tile_pool(name="attsm2", bufs=2) as asm2, \
             tc.tile_pool(name="ostg", bufs=2) as ostg, \
             tc.tile_pool(name="mm", bufs=2, space=PSUM) as mmp, \
             tc.tile_pool(name="sc", bufs=2, space=PSUM) as scp, \
             tc.tile_pool(name="av", bufs=2, space=PSUM) as avp:

            # ---------------- persistent state ----------------
            qT_sb = pp.tile([128, 4, S], bf16)       # roped+normed q
            kT_sb = pp.tile([128, S], bf16)          # roped+normed k
            g_sb = pp.tile([128, 4, S], bf16)        # sigmoid(gate)
            v_sb = pp.tile([128, KC, KVH, HD + 1], bf16)  # natural V + ones
            og_sb = pp.tile([128, 4, S], bf16)       # gated attn out
            wo_sb = pp.tile([128, 4, KC, 128], bf16)
            wq_sb = pp.tile([128, KC, QD], bf16)
            wk_sb = pp.tile([128, KC, KD], bf16)
            wv_sb = pp.tile([128, KC, KD], bf16)
            wg_sb = pp.tile([128, KC, QD], bf16)
            csA_q = pp.tile([128, S], bf16)
            csB_q = pp.tile([128, S], bf16)
            csA_k = pp.tile([128, S], bf16)
            csB_k = pp.tile([128, S], bf16)

            # first block's activations + q-path weights first so the PE
            # can start ASAP; h blocks ride the scalar queue so they don't
            # serialize behind the weight DMAs on sync.
            ha0 = hp.tile([128, 8, 512], bf16, tag="hblk")
            hb0 = hp.tile([128, 8, 512], bf16, tag="hblk")
            wq_view = wqT.rearrange("(c p) m -> p c m", p=128)
            nc.sync.dma_start(out=wq_sb[:, :, 0:128],
                              in_=wq_view[:, :, 0:128])
            for ho in range(2):
                nc.scalar.dma_start(
                    out=ha0[:, 4 * ho:4 * ho + 4, :],
                    in_=hT[512 * ho:512 * ho + 512, 0:512].rearrange(
                        "(c p) s -> p c s", p=128))
            for ho in range(2):
                nc.scalar.dma_start(
                    out=hb0[:, 4 * ho:4 * ho + 4, :],
                    in_=hT[1024 + 512 * ho:1536 + 512 * ho, 0:512].rearrange(
                        "(c p) s -> p c s", p=128))
            for m in range(1, 4):
                nc.sync.dma_start(out=wq_sb[:, :, m * 128:(m + 1) * 128],
                                  in_=wq_view[:, :, m * 128:(m + 1) * 128])
            nc.sync.dma_start(out=wk_sb,
                              in_=wkT.rearrange("(c p) m -> p c m", p=128))
            nc.sync.dma_start(out=csA_q, in_=csAq)
            nc.sync.dma_start(out=csB_q, in_=csBq)
            nc.sync.dma_start(out=csA_k, in_=csAk)
            nc.sync.dma_start(out=csB_k, in_=csBk)
            nc.sync.dma_start(out=wv_sb,
                              in_=wvT.rearrange("(c p) m -> p c m", p=128))
            nc.sync.dma_start(out=wg_sb,
                              in_=wgT.rearrange("(c p) m -> p c m", p=128))
            nc.sync.dma_start(out=wo_sb,
                              in_=woT.rearrange("(c p) (mb mm) -> p c mb mm",
                                                p=128, mm=128))

            ident = pp.tile([128, 128], bf16)
            from concourse.masks import make_identity
            make_identity(nc, ident)
            # ones10[:, c, :]: gather selector -- sumsq of block c's two
            # 64-row halves lands on psum partitions 2c / 2c+1 directly.
            ones10 = pp.tile([128, 5, 10], bf16)
            nc.vector.memset(ones10, 0.0)
            for _c in range(5):
                nc.vector.memset(ones10[0:64, _c, 2 * _c:2 * _c + 1], 1.0)
                nc.vector.memset(ones10[64:128, _c, 2 * _c + 1:2 * _c + 2],
                                 1.0)
            # sel10[:, c, :]: broadcast selector -- rb_ps rows 0:64 get
            # rstd_bf[2c], rows 64:128 get rstd_bf[2c+1] (K=10 matmul).
            # Loaded from DRAM: engine writes can't start at odd partitions.
            sel10 = pp.tile([10, 5, 128], bf16)
            nc.sync.dma_start(out=sel10,
                              in_=sel10d.rearrange("p (c m) -> p c m", m=128))
            nc.vector.memset(v_sb[:, :, :, HD:HD + 1], 1.0)
            if debug_dump:
                nc.vector.memset(og_sb, 777.0)

            # per-sb rstd state (persistent; reused each sb serially)
            sqg = pp.tile([16, 512], f32)            # partition-major sumsq
            rstd_bf = pp.tile([16, 512], bf16)       # final 8/sqrt(ms)

            def proj_chunk(ha, hb, w_sb, msl):
                ps = mmp.tile([128, 512], f32, tag="mm")
                for kc in range(KC):
                    h = ha if kc < 8 else hb
                    nc.tensor.matmul(ps, w_sb[:, kc, msl],
                                     h[:, kc % 8, :],
                                     start=(kc == 0), stop=(kc == KC - 1))
                return ps

            Square = mybir.ActivationFunctionType.Square
            dgather = []   # deferred sumsq-gather closures (see rope_block)

            def flush_dg():
                while dgather:
                    dgather.pop(0)()

            def rope_block(ps, csA, csB, c):
                """bf16 rope on sbuf; defers the sumsq gather one proj
                chain so the PE never waits on the ACT square.  The gather
                matmul reuses rows 0:2 of the spent ps bank (data already
                consumed by raw/sq), keeping the mm-pool rotation intact."""
                raw = rp.tile([128, 512], bf16, tag="raw")
                nc.scalar.copy(out=raw, in_=ps)
                sq = rp.tile([128, 512], bf16, tag="sq", bufs=1)
                nc.scalar.activation(out=sq, in_=ps, func=Square)

                def gather():
                    # selector matmul: block c's sumsq on psum rows 2c:2c+2,
                    # zeros elsewhere; accumulate into sqg rows 0:10 (c=0
                    # initializes and adds the EPS bias for every row).
                    nc.tensor.matmul(ps[0:10, :], ones10[:, c, :], sq,
                                     start=True, stop=True)
                    if c == 0:
                        nc.vector.tensor_scalar(out=sqg[0:10, :],
                                                in0=ps[0:10, :],
                                                scalar1=float(HD) * EPS,
                                                scalar2=None,
                                                op0=AluOpType.add)
                    else:
                        nc.vector.tensor_tensor(out=sqg[0:10, :],
                                                in0=sqg[0:10, :],
                                                in1=ps[0:10, :],
                                                op=AluOpType.add)
                dgather.append(gather)
                rot = rp.tile([128, 512], bf16, tag="rot", bufs=1)
                nc.vector.stream_shuffle(rot, raw, ROT_MASK)
                t1 = rp.tile([128, 512], bf16, tag="t1")
                nc.vector.tensor_tensor(out=t1, in0=raw, in1=csA,
                                        op=AluOpType.mult)
                t2 = rp.tile([128, 512], bf16, tag="t2")
                nc.vector.tensor_tensor(out=t2, in0=rot, in1=csB,
                                        op=AluOpType.mult)
                qa2 = qap.tile([128, 512], bf16, tag="qa2")
                nc.vector.tensor_add(qa2, t1, t2)
                return qa2

            def rstd_chain():
                """sqg[0:10] -> rstd_bf[0:10] (=8/sqrt(ms), bf16)"""
                ms = sqg[0:10, :]
                ms_i = ms.bitcast(i32)
                sh = sqp.tile([16, 512], i32, tag="sh")
                nc.vector.tensor_scalar(out=sh[0:10], in0=ms_i, scalar1=1,
                                        scalar2=None,
                                        op0=AluOpType.logical_shift_right)
                y = sqp.tile([16, 512], f32, tag="y")
                y_i = y.bitcast(i32)
                # y0 bits = MAGIC - (i >> 1)
                nc.vector.tensor_scalar(out=y_i[0:10], in0=sh[0:10],
                                        scalar1=-1, scalar2=MAGIC,
                                        op0=AluOpType.mult,
                                        op1=AluOpType.add)
                for _ in range(2):   # Newton iterations for rsqrt
                    a = sqp.tile([16, 512], f32, tag="nra")
                    nc.vector.tensor_tensor(out=a[0:10], in0=y[0:10],
                                            in1=y[0:10], op=AluOpType.mult)
                    nc.vector.tensor_tensor(out=a[0:10], in0=a[0:10],
                                            in1=ms, op=AluOpType.mult)
                    nc.vector.tensor_scalar(out=a[0:10], in0=a[0:10],
                                            scalar1=-0.5, scalar2=1.5,
                                            op0=AluOpType.mult,
                                            op1=AluOpType.add)
                    nc.vector.tensor_tensor(out=y[0:10], in0=y[0:10],
                                            in1=a[0:10], op=AluOpType.mult)
                nc.vector.tensor_scalar(out=rstd_bf[0:10], in0=y[0:10],
                                        scalar1=float(HD) ** 0.5, scalar2=None,
                                        op0=AluOpType.mult)

            # -------- proj work units (emitted lazily as PE filler) --------
            def proj_units(sb, ha, hb):
                """Full qkv/gate proj + rstd for block sb, as closures.
                Gate chains sit between rstd_chain and the applies so the
                serial rstd DVE chain hides behind ~11us of PE work."""
                s0 = sb * 512
                ssl = slice(s0, s0 + 512)
                qa_list = [None] * 4
                ka2_box = [None]

                def uq(m):
                    def f():
                        ps = proj_chunk(ha, hb, wq_sb,
                                        slice(m * 128, (m + 1) * 128))
                        flush_dg()
                        qa_list[m] = rope_block(ps, csA_q[:, ssl],
                                                csB_q[:, ssl], m)
                    return f

                def uk():
                    psk = proj_chunk(ha, hb, wk_sb, slice(0, 128))
                    flush_dg()
                    ka2_box[0] = rope_block(psk, csA_k[:, ssl],
                                            csB_k[:, ssl], 4)

                def uv():
                    psv = proj_chunk(ha, hb, wv_sb, slice(0, 128))
                    flush_dg()
                    vt = rp.tile([128, 512], bf16, tag="t2")
                    nc.vector.tensor_copy(out=vt, in_=psv)
                    for ss in range(4):
                        tp = mmp.tile([128, 128], bf16, tag="mm")
                        nc.tensor.transpose(tp, vt[:, ss * 128:(ss + 1) * 128],
                                            ident)
                        chunk = sb * 4 + ss
                        nc.vector.tensor_copy(out=v_sb[:, chunk, 0, 0:HD],
                                              in_=tp[:, 0:64])
                        nc.vector.tensor_copy(out=v_sb[:, chunk, 1, 0:HD],
                                              in_=tp[:, 64:128])

                def ug(m):
                    def f():
                        ps = proj_chunk(ha, hb, wg_sb,
                                        slice(m * 128, (m + 1) * 128))
                        th = rp.tile([128, 512], bf16, tag="t1")
                        nc.scalar.activation(out=th, in_=ps, func=Tanh,
                                             scale=0.5)
                        nc.vector.tensor_scalar(out=g_sb[:, m, ssl], in0=th,
                                                scalar1=0.5, scalar2=0.5,
                                                op0=AluOpType.mult,
                                                op1=AluOpType.add)
                    return f

                def uapply(c):
                    def f():
                        rb_ps = mmp.tile([128, 512], f32, tag="mm")
                        nc.tensor.matmul(rb_ps, sel10[:, c, :],
                                         rstd_bf[0:10, :],
                                         start=True, stop=True)
                        if c < 4:
                            r = (c // 2) * 64
                            cb = 2 * (c % 2)
                            nc.vector.tensor_tensor(
                                out=qT_sb[r:r + 64, cb, ssl],
                                in0=qa_list[c][0:64, :], in1=rb_ps[0:64, :],
                                op=AluOpType.mult)
                            nc.vector.tensor_tensor(
                                out=qT_sb[r:r + 64, cb + 1, ssl],
                                in0=qa_list[c][64:128, :],
                                in1=rb_ps[64:128, :],
                                op=AluOpType.mult)
                        else:
                            nc.vector.tensor_tensor(out=kT_sb[:, ssl],
                                                    in0=ka2_box[0],
                                                    in1=rb_ps,
                                                    op=AluOpType.mult)
                    return f

                # pre: everything attn(sb) slot 0 depends on (kT, qT, v);
                # uv hosts the k-block's deferred gather; g0/g2/g1 hide the
                # serial rstd chain before the applies.  late: g3 (needed
                # only by the p=1 drains) gives the attention tail PE work.
                pre = [uq(0), uq(1), uq(2), uq(3), uk, uv, rstd_chain,
                       ug(0), ug(2), ug(1), uapply(4)]
                pre += [uapply(c) for c in range(4)]
                late = [ug(3)]
                return pre, late

            def oproj_units(sb):
                ssl = slice(sb * 512, (sb + 1) * 512)

                def uo(m):
                    def f():
                        po = mmp.tile([128, 512], f32, tag="mm")
                        for oc in range(4):
                            nc.tensor.matmul(po, wo_sb[:, oc, m, :],
                                             og_sb[:, oc, ssl],
                                             start=(oc == 0), stop=(oc == 3))
                        stg = ostg.tile([128, 512], bf16, tag="stg")
                        if m % 2 == 0:
                            nc.scalar.copy(out=stg, in_=po)
                        else:
                            nc.vector.tensor_copy(out=stg, in_=po)
                        nc.sync.dma_start(
                            out=outT[m * 128:(m + 1) * 128, ssl],
                            in_=stg)
                    return f
                return [uo(m) for m in range(KC)]

            # ---------------- attention (per seq block) ----------------
            def attn_sb(sb, early, late):
                """Flat slot stream over 4 (duo-pair, q-head) passes with
                causal trim + fused exp.  AV lags scores by LAG slots and
                carries across pass boundaries; softmax drains are deferred
                into the next pass so the PE pipeline never collapses at a
                pass edge.  early: fillers whose results the next attn
                block needs immediately -- paced to finish a few slots
                before the end; late: dependency-free fillers (oproj) paced
                through the flush/drain tail."""
                s0 = sb * 512
                nkc = 4 * (sb + 1)
                LAG = 2
                passes = [(p, hh) for p in (0, 1) for hh in (0, 1)]
                total = 4 * nkc
                flush = 4
                ne, nl = len(early), len(late)
                eden = max(1, total - 4)
                state = {"slot": 0, "efill": 0, "lfill": 0}
                pstate = {}
                pend_av = []      # (due_slot, pass_idx, t) FIFO
                pend_drain = []   # (due_slot, pass_idx, kvi, phase) FIFO

                def pace():
                    state["slot"] += 1
                    # ceil pacing: a short early list must still empty well
                    # before its in-loop consumers (g3 feeds the p=1 drains)
                    want = min(ne, -(-ne * state["slot"] // eden))
                    while state["efill"] < want:
                        early[state["efill"]]()
                        state["efill"] += 1
                    want = nl * state["slot"] // (total + flush)
                    while state["lfill"] < want:
                        late[state["lfill"]]()
                        state["lfill"] += 1

                def emit_av(pi, t):
                    st = pstate[pi]
                    tl = t - 4 * sb
                    qoff = 128 * tl if tl > 0 else 0
                    qsl = slice(qoff, 512)
                    pr = st["probs"].pop(t)
                    for kvi in range(2):
                        nc.tensor.matmul(
                            st["av"][kvi][:, qsl],
                            v_sb[:, t, kvi, :],
                            pr[:, kvi, qsl],
                            start=(t == 0), stop=(t == nkc - 1))

                def emit_drain1(pi, kvi):
                    """phase 1: the two av reads (cast-copy + psum-direct
                    recip) -- after these the av banks are reusable."""
                    st = pstate[pi]
                    av_t = st["av"][kvi]
                    p, hh = passes[pi]
                    rsl = slice(hh * 64, hh * 64 + 64)
                    t64 = asm2.tile([128, 512], bf16, tag="t")
                    nc.vector.tensor_copy(out=t64[rsl, :], in_=av_t[0:64, :])
                    dn = asm.tile([1, 512], f32, tag="dn", bufs=1)
                    nc.vector.tensor_copy(out=dn, in_=av_t[64:65, :])
                    recip = asm.tile([1, 512], f32, tag="recip")
                    nc.vector.reciprocal_approx_fast(out=recip, in_=dn)
                    st[("d1", kvi)] = (t64, recip)

                def emit_drain2(pi, kvi):
                    """phase 2: og = (av/denom)*gate off the critical path."""
                    p, hh = passes[pi]
                    st = pstate[pi]
                    t64, recip = st.pop(("d1", kvi))
                    duo = p + 2 * kvi
                    rsl = slice(hh * 64, hh * 64 + 64)
                    rb16 = asm.tile([1, 512], bf16, tag="rb16")
                    nc.vector.tensor_copy(out=rb16, in_=recip)
                    rbv = asm.tile([128, 512], bf16, tag="rbv")
                    nc.gpsimd.partition_broadcast(rbv, rb16)
                    t2 = asm2.tile([128, 512], bf16, tag="t2")
                    nc.vector.tensor_tensor(out=t2[rsl, :], in0=t64[rsl, :],
                                            in1=rbv[rsl, :],
                                            op=AluOpType.mult)
                    nc.vector.tensor_tensor(
                        out=og_sb[rsl, duo, s0:s0 + 512],
                        in0=t2[rsl, :],
                        in1=g_sb[rsl, duo, s0:s0 + 512],
                        op=AluOpType.mult)
                    if debug_dump and sb == SB - 1:
                        di = pi * 2 + kvi
                        nc.sync.dma_start(out=dbg["t64"][:, di, :], in_=t64)
                        nc.sync.dma_start(out=dbg["rbv"][:, di, :], in_=rbv)
                        nc.sync.dma_start(out=dbg["t2"][:, di, :], in_=t2)
                        nc.sync.dma_start(
                            out=dbg["ogearly"][rsl, duo, :],
                            in_=og_sb[rsl, duo, s0:s0 + 512])

                for s in range(total + flush):
                    if s < total:
                        pi, t = divmod(s, nkc)
                        if t == 0:
                            p, hh = passes[pi]
                            pstate[pi] = {
                                "qc": 2 * p + hh,
                                "probs": {},
                                "av": [avp.tile([65, 512], f32, tag="av",
                                                name=f"av_{pi}_{i}")
                                       for i in range(2)],
                            }
                        st = pstate[pi]
                        tl = t - 4 * sb
                        qoff = 128 * tl if tl > 0 else 0
                        qsl = slice(qoff, 512)
                        sc2 = scp.tile([128, 2, 512], f32, tag="sc")
                        for kvi in range(2):
                            r0 = 64 * kvi
                            nc.tensor.matmul(
                                sc2[:, kvi, qsl],
                                kT_sb[r0:r0 + 64, t * 128:(t + 1) * 128],
                                qT_sb[r0:r0 + 64, st["qc"],
                                      s0 + qoff:s0 + 512],
                                start=True, stop=True)
                        pr = prp.tile([128, 2, 512], bf16, tag="probs")
                        if (s % 4) in EXP_DVE:
                            nc.vector.tensor_scalar(
                                out=pr[:, :, qsl].bitcast(i16),
                                in0=sc2[:, :, qsl],
                                scalar1=EXP_A, scalar2=EXP_B,
                                op0=AluOpType.mult, op1=AluOpType.add)
                        else:
                            nc.scalar.activation(out=pr[:, :, qsl],
                                                 in_=sc2[:, :, qsl],
                                                 func=Exp, scale=SCALE)
                        if tl >= 0:
                            for kvi in range(2):
                                nc.gpsimd.affine_select(
                                    out=pr[:, kvi, qoff:qoff + 128],
                                    in_=pr[:, kvi, qoff:qoff + 128],
                                    compare_op=mybir.AluOpType.is_ge,
                                    fill=0.0, base=0, channel_multiplier=-1,
                                    pattern=[[1, 128]])
                        st["probs"][t] = pr
                        # first AVs of a pass wait two extra slots so the
                        # previous pass's drain reads land well before the
                        # av-bank start=True reuse (PE-W vs DVE-R hazard).
                        pend_av.append((s + LAG + (2 if t <= 1 else 0),
                                        pi, t))
                        if t == nkc - 1:
                            # drain phase 1 right after this pass's last AV
                            pend_drain.append((s + LAG + 1, pi, 0, 1))
                            pend_drain.append((s + LAG + 1, pi, 1, 1))
                            pend_drain.append((s + LAG + 2, pi, 0, 2))
                            pend_drain.append((s + LAG + 2, pi, 1, 2))
                    # deferred drains first (they free av banks), then AVs
                    while pend_drain and pend_drain[0][0] <= s:
                        _, pi2, kvi, ph = pend_drain.pop(0)
                        (emit_drain1 if ph == 1 else emit_drain2)(pi2, kvi)
                    while pend_av and pend_av[0][0] <= s:
                        _, pi2, t2 = pend_av.pop(0)
                        emit_av(pi2, t2)
                    pace()
                while pend_drain:
                    _, pi2, kvi, ph = pend_drain.pop(0)
                    (emit_drain1 if ph == 1 else emit_drain2)(pi2, kvi)
                while pend_av:
                    _, pi2, t2 = pend_av.pop(0)
                    emit_av(pi2, t2)
                while state["efill"] < ne:
                    early[state["efill"]]()
                    state["efill"] += 1
                while state["lfill"] < nl:
                    late[state["lfill"]]()
                    state["lfill"] += 1

            # ================= fused pipeline =================
            h_tiles = {0: (ha0, hb0)}
            pre0, late0 = proj_units(0, ha0, hb0)
            for u in pre0:
                u()
            late_units = {0: late0}
            for sb in range(SB):
                if sb < SB - 1:
                    s1 = (sb + 1) * 512
                    ha = hp.tile([128, 8, 512], bf16, tag="hblk")
                    hb = hp.tile([128, 8, 512], bf16, tag="hblk")
                    nc.scalar.dma_start(
                        out=ha,
                        in_=hT[0:1024, s1:s1 + 512].rearrange(
                            "(c p) s -> p c s", p=128))
                    nc.scalar.dma_start(
                        out=hb,
                        in_=hT[1024:2048, s1:s1 + 512].rearrange(
                            "(c p) s -> p c s", p=128))
                    h_tiles[sb + 1] = (ha, hb)
                early = list(late_units[sb])
                late = []
                if sb > 0:
                    late = oproj_units(sb - 1)
                if sb < SB - 1:
                    pre_n, late_n = proj_units(sb + 1, *h_tiles[sb + 1])
                    early = early + pre_n
                    late_units[sb + 1] = late_n
                attn_sb(sb, early, late)
            for u in oproj_units(SB - 1):
                u()

            if debug_dump:
                nc.sync.dma_start(out=dbg["q"], in_=qT_sb)
                nc.sync.dma_start(out=dbg["k"], in_=kT_sb)
                nc.sync.dma_start(out=dbg["g"], in_=g_sb)
                nc.sync.dma_start(out=dbg["v"], in_=v_sb)
                nc.sync.dma_start(out=dbg["og"], in_=og_sb)
                nc.sync.dma_start(out=dbg["rstd"], in_=rstd_bf)

    nc.compile()
    return nc


def _host_prep(hidden_states, cos, sin, Wq, Wk, Wv, Wg, Wo, q_norm_w, k_norm_w):
    """Build per-core input maps."""

    def cs_tables(cos_b, sin_b, w):
        # csA/csB [128, S]: row p -> head-local dim d = p % 64
        A = np.empty((128, S), np.float32)
        Bt = np.empty((128, S), np.float32)
        cosT = cos_b.T  # [32, S]
        sinT = sin_b.T
        for blk in (0, 64):
            A[blk + 0:blk + 32] = cosT * w[0:32, None]
            A[blk + 32:blk + 64] = w[32:64, None]
            Bt[blk + 0:blk + 16] = -sinT[0:16] * w[16:32, None]
            Bt[blk + 16:blk + 32] = sinT[16:32] * w[0:16, None]
            Bt[blk + 32:blk + 64] = 0.0
        return A.astype(BF16), Bt.astype(BF16)

    in_maps = []
    for c in range(NCORES):
        b, g = c // 4, c % 4
        qs = slice(g * QD, (g + 1) * QD)
        ks = slice(g * KD, (g + 1) * KD)
        csA_q, csB_q = cs_tables(cos[b], sin[b], np.asarray(q_norm_w))
        csA_k, csB_k = cs_tables(cos[b], sin[b], np.asarray(k_norm_w))
        in_maps.append({
            "hT": np.ascontiguousarray(hidden_states[b].T).astype(BF16),
            "wqT": np.ascontiguousarray(Wq[qs].T).astype(BF16),
            "wkT": np.ascontiguousarray(Wk[ks].T).astype(BF16),
            "wvT": np.ascontiguousarray(Wv[ks].T).astype(BF16),
            "wgT": np.ascontiguousarray(Wg[qs].T).astype(BF16),
            "woT": np.ascontiguousarray(Wo[:, qs].T).astype(BF16),
            "csAq": csA_q, "csBq": csB_q, "csAk": csA_k, "csBk": csB_k,
            "sel10d": SEL10,
        })
    return in_maps


def kernel(hidden_states, cos, sin, Wq, Wk, Wv, Wg, Wo, q_norm_w, k_norm_w):
    from concourse import bass_utils

    if "nc" not in _CACHE:
        _CACHE["nc"] = _build_bass()
    nc = _CACHE["nc"]

    in_maps = _host_prep(hidden_states, cos, sin, Wq, Wk, Wv, Wg, Wo,
                         q_norm_w, k_norm_w)

    trace = bool(int(os.environ.get("KERNEL_TRACE", "0")))
    kwargs = {}
    if trace:
        # the agent image's antenv lacks axon_hooks; recreate it from the
        # boot helper so run_bass_kernel_spmd(trace=True) can NTFF-profile
        try:
            import antenv.axon_hooks  # noqa: F401
        except ImportError:
            import types
            sys.path.insert(0, "/root/.axon_site")
            from trn_agent_boot.trn_boot import _ntff_profile_via_ctypes
            hook = _ntff_profile_via_ctypes("/opt/axon/libaxon_pjrt.so")
            mod = types.ModuleType("antenv.axon_hooks")
            mod.get_axon_ntff_profile_hook = lambda: hook
            sys.modules["antenv.axon_hooks"] = mod
        tmpdir = os.environ.get("KERNEL_TRACE_DIR") or None
        kwargs = dict(trace=True, tmpdir=tmpdir)
    res = bass_utils.run_bass_kernel_spmd(nc, in_maps,
                                          core_ids=list(range(NCORES)),
                                          **kwargs)
    if trace and res.exec_time_ns is not None:
        print(f"HW exec time: {res.exec_time_ns} ns")
        _CACHE["exec_time_ns"] = res.exec_time_ns

    out = np.zeros((B, S, HID), np.float32)
    for c in range(NCORES):
        b = c // 4
        out[b] += res.results[c]["outT"].astype(np.float32).T
    return out


if __name__ == "__main__":
    rng = np.random.default_rng(0)
    hs = rng.standard_normal((B, S, HID), dtype=np.float32)
    cos = rng.random((B, S, ROPE), dtype=np.float32)
    sin = rng.random((B, S, ROPE), dtype=np.float32)
    out = kernel(hidden_states=hs, cos=cos, sin=sin,
                 Wq=rng.standard_normal((NH * HD, HID), dtype=np.float32) * 0.02,
                 Wk=rng.standard_normal((NKV * HD, HID), dtype=np.float32) * 0.02,
                 Wv=rng.standard_normal((NKV * HD, HID), dtype=np.float32) * 0.02,
                 Wg=rng.standard_normal((NH * HD, HID), dtype=np.float32) * 0.02,
                 Wo=rng.standard_normal((HID, NH * HD), dtype=np.float32) * 0.02,
                 q_norm_w=np.ones(HD, np.float32),
                 k_norm_w=np.ones(HD, np.float32))
    print(out.shape, out.dtype)
